# revision 25
# baseline (speedup 1.0000x reference)
"""Trainium2 Bass kernel for nn_Decoder: per-edge bilinear decoder.

  out[e, c] = relu( u[e] @ W'_c @ v[e] ),  W'_c = sum_k Wc[k,c] W[k]
  u = user_inputs[user_indices], v = item_inputs[item_indices]

Strategy (8 NeuronCores, sharded by ITEM block, 6250 items/core):
  - Per core: device precomputes s[i, c*128+d] = sum_f W'_c[d,f] item[i,f]
    for its item shard (49 matmul chunks vs uploaded W2[f, (c,d)]), kept in
    SBUF in gather-rank layout.
  - Host sorts the core's edges by (user_block, item) and packs them into
    8-edge SLOTS, each slot belonging to one item (runs padded to slot
    multiples; pad gathers row 0). User ids span 100k > int16, so two
    65536-row blocks are addressed with SIGNED int16 gather offsets
    (bases 32768 / 98304).
  - Steady state per 512-slot batch (4096 edges):
      * dma_gather(transpose=True) -> uT [128 d, 4096 e] (no PE transpose)
      * SBUF-source dma_gather(transpose=True) -> sB [128 d, 5 c, 512 slots]
      * per slot j: matmul(lhsT=sB[:,:,j] [d,5], rhs=uT slot [d,8])
        -> PSUM [5, 8] packed 3 stripes x 64 col-groups per bank
      * ScalarE relu-copies each bank [69, 512] PSUM->SBUF, 3 stripe DMAs
        to DRAM. VectorE is completely idle; host un-permutes + final dtype.
"""
import sys
import os
import math
import functools

for _p in ("/opt/trn_rl_repo", "/root/.axon_site/_ro/trn_rl_repo"):
    if os.path.isdir(_p) and _p not in sys.path:
        sys.path.insert(0, _p)

import numpy as np
import ml_dtypes

import concourse.bass as bass
import concourse.bacc as bacc
from concourse import mybir
from concourse.tile import TileContext
from concourse.bass_utils import run_bass_kernel_spmd

bf16 = ml_dtypes.bfloat16
F32 = mybir.dt.float32
BF16 = mybir.dt.bfloat16
I16 = mybir.dt.int16

N_USERS, N_ITEMS, D, E, K, C = 100000, 50000, 128, 1000000, 8, 5
NCORES = 8
P = 128
IPC = N_ITEMS // NCORES          # 6250 items per core
IPCP = 6272                      # rounded up to 128 (gather num_idxs)
NCHUNK = IPCP // P               # 49 s-precompute chunks
S = 8                            # edges per slot
SPB = 512                        # slots per batch
EPB = SPB * S                    # 4096 edges per batch
UB_SPLIT = 65536                 # user block boundary
# signed-int16 gather bases. BASE1 must leave >=32768 rows in the sliced
# table view: a slice smaller than the idx reach crashes the gather engine.
BASE0, BASE1 = 32768, 67232
CF = C * D                       # 640
# slot -> psum packing: 3 stripes (base part 0/32/64) x 64 col-groups -> 192
# slots per bank; banks 0,1 hold 192, bank 2 holds 128. Each bank stages out
# as 3 full 512-col stripes -> 4608 output columns per batch.
OUT_PB = 3 * 3 * 512
# the last 8 outputs of a 512-idx transposed gather can land AFTER its
# completion semaphore (hw tail race) -> the last slot of each 64-slot
# u-gather chunk is always a pad slot. 63*8 usable slots per batch.
NGC = 8                          # u-gather chunks per batch (512 idx each)
GCH = 512
USABLE = 63 * NGC                # usable slots per batch


def _slot_outcol(j):
    """psum/out column base for in-batch slot j (vectorized)."""
    j = np.asarray(j)
    bank = np.minimum(j // 192, 2)
    jj = j - bank * 192
    stripe = jj % 3
    cg = jj // 3
    return bank * 1536 + stripe * 512 + cg * 8


@functools.lru_cache(maxsize=4)
def _build_program(nb0: int, nb1: int, repeats: int = 1):
    nbtot = nb0 + nb1
    nslot = nbtot * SPB
    uidx_cols = nslot * S // 16
    sidx_cols = nslot // 16

    nc = bacc.Bacc("TRN2", target_bir_lowering=False, debug=False,
                   num_devices=NCORES, num_swdge_queues=2)

    ut_d = nc.declare_dram_parameter("ut", [N_USERS, D], BF16, isOutput=False)
    it_d = nc.declare_dram_parameter("it", [IPCP, D], BF16, isOutput=False)
    w2_d = nc.declare_dram_parameter("w2", [P, CF], BF16, isOutput=False)
    uidx_d = nc.declare_dram_parameter("uidx", [P, uidx_cols], I16, isOutput=False)
    sidx_d = nc.declare_dram_parameter("sidx", [P, sidx_cols], I16, isOutput=False)
    iidx_d = nc.declare_dram_parameter("iidx", [P, IPCP // 16], I16, isOutput=False)
    id_d = nc.declare_dram_parameter("ident", [P, P], BF16, isOutput=False)
    out_d = nc.declare_dram_parameter("outp", [C, nbtot * OUT_PB], F32,
                                      isOutput=True)

    with TileContext(nc) as tc:
        with (
            tc.tile_pool(name="const", bufs=1) as const,
            tc.tile_pool(name="stmp", bufs=2) as stmp,
            tc.tile_pool(name="ug", bufs=3) as ugp,
            tc.tile_pool(name="sg", bufs=3) as sgp,
            tc.tile_pool(name="stg", bufs=4) as stp,
            tc.tile_pool(name="psA", bufs=1, space="PSUM") as psA,
            tc.tile_pool(name="psB", bufs=2, space="PSUM") as psB,
        ):
            uidx_sb = const.tile([P, uidx_cols], I16)
            nc.sync.dma_start(out=uidx_sb[:], in_=uidx_d[:])
            sidx_sb = const.tile([P, sidx_cols], I16)
            nc.sync.dma_start(out=sidx_sb[:], in_=sidx_d[:])
            iidx_sb = const.tile([P, IPCP // 16], I16)
            nc.sync.dma_start(out=iidx_sb[:], in_=iidx_d[:])
            w2_sb = const.tile([P, CF], BF16)
            nc.sync.dma_start(out=w2_sb[:], in_=w2_d[:])
            id_sb = const.tile([P, P], BF16)
            nc.sync.dma_start(out=id_sb[:], in_=id_d[:])
            s_sb = const.tile([P, NCHUNK * CF], BF16)   # rank r = chunk

            tc.strict_bb_all_engine_barrier()

            # itemT via non-transposed gathers + PE transposes (transposed
            # gathers have a tail race; these are race-free)
            IT_CH = [1024] * 6 + [128]
            it_raw = []
            _off = 0
            for q, n in enumerate(IT_CH):
                itR = const.tile([P, n // P, P], BF16, name=f"itR{q}")
                nc.gpsimd.dma_gather(
                    out_ap=itR[:], in_ap=it_d[0:, :],
                    idxs_ap=iidx_sb[:, _off // 16:(_off + n) // 16],
                    num_idxs=n, num_idxs_reg=n, elem_size=D,
                )
                it_raw.append(itR)
                _off += n
            itT_sb = const.tile([P, IPCP], BF16)
            for ch in range(NCHUNK):
                pt = psA.tile([P, P], F32, name="pb")   # reuse pb's bank
                pt_bf = pt[:].bitcast(BF16)
                nc.tensor.transpose(out=pt_bf[:, 0:P],
                                    in_=it_raw[ch // 8][:, ch % 8, :],
                                    identity=id_sb[:])
                nc.scalar.copy(out=itT_sb[:, ch * P:(ch + 1) * P],
                               in_=pt_bf[:, 0:P])
            # s rows: s[i, c*128+d]; chunk of 128 items at a time
            for ch in range(NCHUNK):
                pa = psA.tile([P, 512], F32)
                pb = psA.tile([P, P], F32)
                rhs_i = itT_sb[:, ch * P:(ch + 1) * P]
                nc.tensor.matmul(out=pa[:], lhsT=rhs_i, rhs=w2_sb[:, 0:512],
                                 start=True, stop=True)
                nc.tensor.matmul(out=pb[:], lhsT=rhs_i, rhs=w2_sb[:, 512:CF],
                                 start=True, stop=True)
                nc.scalar.copy(out=s_sb[:, ch * CF:ch * CF + 512], in_=pa[:])
                nc.scalar.copy(out=s_sb[:, ch * CF + 512:(ch + 1) * CF], in_=pb[:])

            for b in [b for _ in range(repeats) for b in range(nbtot)]:
                base = BASE0 if b < nb0 else BASE1
                u_tiles = []
                for q in range(NGC):
                    uT = ugp.tile([P, 1, GCH], BF16, name=f"uT{q}")
                    c0 = b * (EPB // 16) + q * (GCH // 16)
                    nc.gpsimd.dma_gather(
                        out_ap=uT[:], in_ap=ut_d[base:, :],
                        idxs_ap=uidx_sb[:, c0:c0 + GCH // 16],
                        num_idxs=GCH, num_idxs_reg=GCH, elem_size=D,
                        transpose=True,
                    )
                    u_tiles.append(uT)
                sB = sgp.tile([P, C, SPB], BF16)
                nc.gpsimd.dma_gather(
                    out_ap=sB[:], in_ap=s_sb[:],
                    idxs_ap=sidx_sb[:, b * (SPB // 16):(b + 1) * (SPB // 16)],
                    num_idxs=SPB, num_idxs_reg=SPB, elem_size=CF, transpose=True,
                    queue_num=1, sbuf_tokens_per_rank=P,
                    sbuf_free_dim_per_rank=CF * 2,
                )
                banks = [psB.tile([P, 512], F32, name=f"bankk{kb}")
                         for kb in range(3)]
                for j in range(SPB):
                    bank = min(j // 192, 2)
                    jj = j - bank * 192
                    stripe, cg = jj % 3, jj // 3
                    bp = 32 * stripe
                    jq, jr = divmod(j * S, GCH)
                    nc.tensor.matmul(
                        out=banks[bank][bp:bp + C, cg * S:(cg + 1) * S],
                        lhsT=sB[:, :, j],
                        rhs=u_tiles[jq][:, 0, jr:jr + S],
                        start=True, stop=True,
                    )
                for k in range(3):
                    stage = stp.tile([69, 512], F32)
                    nc.scalar.activation(out=stage[:], in_=banks[k][0:69, :],
                                         func=mybir.ActivationFunctionType.Relu)
                    for st in range(3):
                        o0 = b * OUT_PB + k * 1536 + st * 512
                        nc.sync.dma_start(out=out_d[:, o0:o0 + 512],
                                          in_=stage[32 * st:32 * st + C, :])

    nc.compile()
    return nc, nbtot


def _ceil_to(x, m):
    return int(math.ceil(x / m) * m)


def _prep_core(ui, vi, lo):
    """Sort one core's edges into slots. Returns (ns0, ns1, packed) where
    packed defers final placement until global NS0/NS1 are known."""
    sel = np.flatnonzero((vi >= lo) & (vi < lo + IPC))
    u = ui[sel]
    v_loc = (vi[sel] - lo).astype(np.int64)
    ub = (u >= UB_SPLIT).astype(np.int64)
    key = ub * 8192 + v_loc
    order = np.argsort(key, kind="stable")
    sel_s, u_s, key_s, ub_s = sel[order], u[order], key[order], ub[order]
    runs_key, run_counts = np.unique(key_s, return_counts=True)
    run_slots = (run_counts + S - 1) // S
    run_ub = runs_key >= 8192
    ns0 = int(run_slots[~run_ub].sum())
    ns1 = int(run_slots[run_ub].sum())
    return ns0, ns1, (sel_s, u_s, ub_s, runs_key, run_counts, run_slots, run_ub)


def _u2jg(u):
    """usable-slot index -> global slot id b*SPB + j (skips j%64==63)."""
    b, w = np.divmod(u, USABLE)
    c, r = np.divmod(w, 63)
    return b * SPB + c * 64 + r


def _fill_core(packed, ns0_c, nb0, nb1):
    """Place slots given global batch counts; returns (u16, sidx16, orig,
    outcol_per_slot)."""
    sel_s, u_s, ub_s, runs_key, run_counts, run_slots, run_ub = packed
    nslot = (nb0 + nb1) * SPB
    R = len(runs_key)
    run_soff = np.cumsum(run_slots) - run_slots          # usable units
    slot_off = run_soff + np.where(run_ub, nb0 * USABLE - ns0_c, 0)

    run_eoff = np.cumsum(run_counts) - run_counts
    run_id = np.repeat(np.arange(R), run_counts)
    within = np.arange(len(sel_s)) - run_eoff[run_id]
    slotpos = _u2jg(slot_off[run_id] + within // S) * S + within % S

    u16 = np.zeros(nslot * S, np.int16)
    u16[slotpos] = (u_s - np.where(ub_s == 1, BASE1, BASE0)).astype(np.int16)

    sidx = np.zeros(nslot, np.int16)
    tot_slots = int(run_slots.sum())
    spos = np.repeat(slot_off, run_slots) + (
        np.arange(tot_slots) - np.repeat(run_soff, run_slots))
    sidx[_u2jg(spos)] = np.repeat((runs_key % 8192).astype(np.int16), run_slots)

    orig = np.full(nslot * S, -1, np.int64)
    orig[slotpos] = sel_s

    gslot = np.arange(nslot)
    outcol_slot = (gslot // SPB) * OUT_PB + _slot_outcol(gslot % SPB)
    return u16, sidx, orig, outcol_slot


def _wrap16(x, ncols_per_batch, nb):
    """[nb*16*ncols] -> [128, nb*ncols] (16-part wrap, replicated x8)."""
    w = x.reshape(nb, ncols_per_batch, 16).transpose(2, 0, 1).reshape(16, -1)
    return np.ascontiguousarray(np.tile(w, (8, 1)))


def _prepare(user_inputs, item_inputs, user_indices, item_indices,
             weight, weight_classifier, repeats=1):
    user_inputs = np.asarray(user_inputs)
    item_inputs = np.asarray(item_inputs)
    ui = np.asarray(user_indices).astype(np.int64)
    vi = np.asarray(item_indices).astype(np.int64)
    weight = np.asarray(weight, dtype=np.float32)
    wc = np.asarray(weight_classifier, dtype=np.float32)

    # W2[f, c*128+d] = sum_k Wc[k,c] * W[k][d, f]
    w2 = np.einsum("kdf,kc->fcd", weight, wc).reshape(D, CF)
    w2 = np.ascontiguousarray(w2).astype(bf16)
    ut_bf = np.ascontiguousarray(user_inputs.astype(bf16))

    iota = np.arange(IPCP, dtype=np.int16)
    iidx = _wrap16(iota, IPCP // 16, 1)

    cores = []
    ns0s, ns1s = [], []
    for c in range(NCORES):
        ns0, ns1, packed = _prep_core(ui, vi, c * IPC)
        cores.append((ns0, packed))
        ns0s.append(ns0)
        ns1s.append(ns1)
    nb0 = _ceil_to(max(ns0s), USABLE) // USABLE
    nb1 = _ceil_to(max(ns1s), USABLE) // USABLE

    nc, nbtot = _build_program(nb0, nb1, repeats)

    ident = np.eye(P, dtype=np.float32).astype(bf16)
    in_maps, posts = [], []
    for c in range(NCORES):
        ns0_c, packed = cores[c]
        u16, sidx, orig, outcol_slot = _fill_core(packed, ns0_c, nb0, nb1)
        it_shard = np.zeros((IPCP, D), bf16)
        it_shard[:IPC] = item_inputs[c * IPC:(c + 1) * IPC].astype(bf16)
        in_maps.append({
            "ut": ut_bf,
            "it": it_shard,
            "w2": w2,
            "uidx": _wrap16(u16, EPB // 16, nbtot),
            "sidx": _wrap16(sidx, SPB // 16, nbtot),
            "iidx": iidx,
            "ident": ident,
        })
        posts.append((orig, outcol_slot))
    return nc, nbtot, in_maps, posts


def _postprocess(results, posts):
    out = np.empty((E, C), np.float32)
    for c in range(NCORES):
        o = results[c]["outp"]            # [5, nbtot*OUT_PB]
        orig, outcol_slot = posts[c]
        mask = orig >= 0
        pos = np.flatnonzero(mask)
        cols = outcol_slot[pos // S] + pos % S
        out[orig[pos]] = o[:, cols].T
    return out


def kernel(user_inputs, item_inputs, user_indices, item_indices,
           weight, weight_classifier):
    nc, nbtot, in_maps, posts = _prepare(
        user_inputs, item_inputs, user_indices, item_indices,
        weight, weight_classifier)
    results = run_bass_kernel_spmd(nc, in_maps, list(range(NCORES))).results
    return _postprocess(results, posts)
